# revision 7
# baseline (speedup 1.0000x reference)
"""Trainium2 Bass kernel for speculative-decoding rejection sampling.

kernel(**inputs) takes the FULL inputs (B=128 requests x SPEC=8 draft tokens,
V=32000 vocab) and returns the FULL [128, 9] int32 output. The 128 requests
are sharded 16-per-core across 8 NeuronCores (data parallel over requests);
each core keeps full vocab rows for its tokens so argmax / softmax reductions
over vocab stay local. Greedy-request rows only need argmax of the logits;
non-greedy rows only need the softmax denominator; the expensive "recovered
token" argmax over max(p-d,0)/q is computed on-device only for the single
first-rejection row of each non-greedy request, selected with on-device
indirect DMA gathers.
"""
from contextlib import ExitStack

import numpy as np

GREEDY_TEMPERATURE = -1.0
PLACEHOLDER = -1
B, SPEC, V = 128, 8, 32000
NCORES = 8
RPC = B // NCORES      # 16 requests per core
HALF = V // 2          # 16000
CH = 2                 # vocab chunks per half for the streaming pass
CHW = HALF // CH       # 8000
SUB = V // 16          # 2000

_NC_CACHE = {}


def _build():
    import concourse.bass as bass
    import concourse.bacc as bacc
    import concourse.tile as tile
    from concourse import mybir

    F32 = mybir.dt.float32
    I32 = mybir.dt.int32
    U32 = mybir.dt.uint32
    AF = mybir.ActivationFunctionType
    OP = mybir.AluOpType
    AX = mybir.AxisListType

    nc = bacc.Bacc("TRN2", num_devices=8)

    lg_e = nc.declare_dram_parameter("lg", [128, HALF], F32, isOutput=False)
    lr_e = nc.declare_dram_parameter("lr", [128, HALF], F32, isOutput=False)
    dr_e = nc.declare_dram_parameter("dr", [64, V], F32, isOutput=False)
    qr_e = nc.declare_dram_parameter("qr", [8, V], F32, isOutput=False)
    m128_e = nc.declare_dram_parameter("m128", [128, 3], F32, isOutput=False)
    m9_e = nc.declare_dram_parameter("m9", [9, 48], F32, isOutput=False)
    m8_e = nc.declare_dram_parameter("m8", [8, 36], F32, isOutput=False)
    m64_e = nc.declare_dram_parameter("m64", [64, 2], F32, isOutput=False)
    offs_e = nc.declare_dram_parameter("offs", [64, 1], I32, isOutput=False)
    out_e = nc.declare_dram_parameter("out", [9, 16], I32, isOutput=True)

    lr_flat1 = lr_e.rearrange("a b -> (a b)").rearrange("(r s) -> r s", s=1)
    dr_flat1 = dr_e.rearrange("a b -> (a b)").rearrange("(r s) -> r s", s=1)
    lr_sub = lr_e.rearrange("a b -> (a b)").rearrange("(r s) -> r s", s=SUB)
    dr_sub = dr_e.rearrange("a b -> (a b)").rearrange("(r s) -> r s", s=SUB)
    qr_fold = qr_e.rearrange("a b -> (a b)").rearrange("(p s) -> p s", s=SUB)

    with tile.TileContext(nc) as tc, ExitStack() as ctx:
        const = ctx.enter_context(tc.tile_pool(name="const", bufs=1))
        big = ctx.enter_context(tc.tile_pool(name="big", bufs=2))
        lrp = ctx.enter_context(tc.tile_pool(name="lrp", bufs=2))
        keepp = ctx.enter_context(tc.tile_pool(name="keepp", bufs=1))
        small = ctx.enter_context(tc.tile_pool(name="small", bufs=1))
        psum = ctx.enter_context(tc.tile_pool(name="psum", bufs=1, space="PSUM"))

        # ---- big streaming DMAs first (sync/HWDGE queue, FIFO) ----
        lgt = [big.tile([128, CHW], F32, tag="lg", name=f"lgt{_i}") for _i in range(CH)]
        lrt = [lrp.tile([128, CHW], F32, tag="lr", name=f"lrt{_i}") for _i in range(CH)]
        qsb = keepp.tile([128, SUB], F32, tag="qsb")
        nc.sync.dma_start(lgt[0][:], lg_e[:, 0:CHW])
        nc.sync.dma_start(lrt[0][:], lr_e[:, 0:CHW])
        nc.sync.dma_start(qsb[:], qr_fold[:])
        nc.sync.dma_start(lgt[1][:], lg_e[:, CHW:HALF])
        nc.sync.dma_start(lrt[1][:], lr_e[:, CHW:HALF])

        # ---- small metadata on the gpsimd/SWDGE queue ----
        m128 = const.tile([128, 3], F32, tag="m128")
        nc.gpsimd.dma_start(m128[:], m128_e[:])
        off2000_sb, pmfull_sb, choff1_sb = m128[:, 0:1], m128[:, 1:2], m128[:, 2:3]
        m9 = const.tile([9, 48], F32, tag="m9")
        nc.gpsimd.dma_start(m9[:], m9_e[:])
        valid9_sb, bonus_oh_sb, bonusR_sb = m9[:, 0:16], m9[:, 16:32], m9[:, 32:48]
        m8 = const.tile([8, 36], F32, tag="m8")
        nc.gpsimd.dma_start(m8[:], m8_e[:])
        didr8_sb, U9_sb, ones8_sb = m8[:, 0:8], m8[:, 8:17], m8[:, 17:18]
        jiota16_sb, c128_sb = m8[:, 18:34], m8[:, 34:35]
        m64 = const.tile([64, 2], F32, tag="m64")
        nc.gpsimd.dma_start(m64[:], m64_e[:])
        u_sb, didg_sb = m64[:, 0:1], m64[:, 1:2]
        offs_sb = const.tile([64, 1], I32, tag="offs")
        nc.gpsimd.dma_start(offs_sb[:], offs_e[:])

        # early element gathers: target_logits / draft_probs at the draft ids
        ld_sb = small.tile([64, 1], F32, tag="ld")
        nc.gpsimd.indirect_dma_start(
            out=ld_sb[:], out_offset=None, in_=lr_flat1[:],
            in_offset=bass.IndirectOffsetOnAxis(ap=offs_sb[:, :1], axis=0))
        pd_sb = small.tile([64, 1], F32, tag="pd")
        nc.gpsimd.indirect_dma_start(
            out=pd_sb[:], out_offset=None, in_=dr_flat1[:],
            in_offset=bass.IndirectOffsetOnAxis(ap=offs_sb[:, :1], axis=0))

        # -1/q = -sign(q)/|q| via exp(-ln(max(|q|,eps))) + one Newton step
        qa = keepp.tile([128, SUB], F32, tag="qa")
        nc.scalar.activation(qa[:], qsb[:], AF.Abs)
        nc.vector.tensor_scalar(out=qa[:], in0=qa[:], scalar1=1e-38, scalar2=None, op0=OP.max)
        lnq = keepp.tile([128, SUB], F32, tag="lnq")
        nc.scalar.activation(lnq[:], qa[:], AF.Ln)
        r0 = keepp.tile([128, SUB], F32, tag="r0")
        nc.scalar.activation(r0[:], lnq[:], AF.Exp, scale=-1.0)
        t0q = keepp.tile([128, SUB], F32, tag="t0q")
        nc.vector.tensor_tensor(out=t0q[:], in0=qa[:], in1=r0[:], op=OP.mult)
        rqs = keepp.tile([128, SUB], F32, tag="rqs")
        nc.vector.scalar_tensor_tensor(out=rqs[:], in0=t0q[:], scalar=2.0, in1=r0[:], op0=OP.subtract, op1=OP.mult)
        sgn = keepp.tile([128, SUB], F32, tag="sgn")
        nc.vector.tensor_scalar(out=sgn[:], in0=qsb[:], scalar1=0.0, scalar2=2.0, op0=OP.is_gt, op1=OP.mult)
        nc.vector.tensor_scalar(out=sgn[:], in0=sgn[:], scalar1=1.0, scalar2=None, op0=OP.subtract)
        nc.vector.tensor_tensor(out=rqs[:], in0=rqs[:], in1=sgn[:], op=OP.mult)

        eld = small.tile([64, 1], F32, tag="eld")
        nc.scalar.activation(eld[:], ld_sb[:], AF.Exp)

        # phase 1: DVE max/argmax on greedy rows, ACT exp+accum on non-greedy
        gmax8 = small.tile([128, 8 * CH], F32, tag="gmax8")
        gidx8 = small.tile([128, 8 * CH], U32, tag="gidx8")
        zacc = small.tile([128, CH], F32, tag="zacc")
        for k in range(CH):
            nc.vector.max(out=gmax8[:, 8 * k:8 * k + 8], in_=lgt[k][:])
            nc.vector.max_index(gidx8[:, 8 * k:8 * k + 8], gmax8[:, 8 * k:8 * k + 8], lgt[k][:])
            nc.scalar.activation(lrt[k][:], lrt[k][:], AF.Exp, accum_out=zacc[:, k:k + 1])

        # per-partition argmax combine across the 2 chunks (ties -> chunk 0)
        h2 = small.tile([128, 1], F32, tag="h2")
        nc.vector.tensor_tensor(out=h2[:], in0=gmax8[:, 8:9], in1=gmax8[:, 0:1], op=OP.is_gt)
        gmax_p = small.tile([128, 1], F32, tag="gmax_p")
        nc.vector.tensor_tensor(out=gmax_p[:], in0=gmax8[:, 0:1], in1=gmax8[:, 8:9], op=OP.max)
        i0 = small.tile([128, 2], F32, tag="i0")
        nc.vector.tensor_copy(i0[:], gidx8[:, 0:16:8])            # cast u32->f32
        parg = small.tile([128, 1], F32, tag="parg")
        nc.vector.tensor_scalar(out=parg[:], in0=i0[:, 1:2], scalar1=float(CHW), scalar2=None, op0=OP.add)
        nc.vector.tensor_tensor(out=parg[:], in0=parg[:], in1=i0[:, 0:1], op=OP.subtract)
        nc.vector.tensor_tensor(out=parg[:], in0=parg[:], in1=h2[:], op=OP.mult)
        nc.vector.tensor_tensor(out=parg[:], in0=parg[:], in1=i0[:, 0:1], op=OP.add)

        # combine the two half-row partitions of each greedy slot
        pk = small.tile([128, 2], F32, tag="pk")
        nc.vector.tensor_copy(pk[:, 0:1], gmax_p[:])
        nc.vector.tensor_copy(pk[:, 1:2], parg[:])
        pk2 = small.tile([64, 4], F32, tag="pk2")
        nc.scalar.dma_start(pk2[:], pk[:])
        hsel = small.tile([64, 1], F32, tag="hsel")
        nc.vector.tensor_tensor(out=hsel[:], in0=pk2[:, 2:3], in1=pk2[:, 0:1], op=OP.is_gt)
        amx = small.tile([64, 1], F32, tag="amx")
        nc.vector.tensor_scalar(out=amx[:], in0=pk2[:, 3:4], scalar1=float(HALF), scalar2=None, op0=OP.add)
        nc.vector.tensor_tensor(out=amx[:], in0=amx[:], in1=pk2[:, 1:2], op=OP.subtract)
        nc.vector.tensor_tensor(out=amx[:], in0=amx[:], in1=hsel[:], op=OP.mult)
        nc.vector.tensor_tensor(out=amx[:], in0=amx[:], in1=pk2[:, 1:2], op=OP.add)
        acc_g = small.tile([64, 1], F32, tag="acc_g")
        nc.vector.tensor_tensor(out=acc_g[:], in0=didg_sb, in1=amx[:], op=OP.is_equal)

        # softmax denominators per slot; acceptance tests
        zsum = small.tile([128, 1], F32, tag="zsum")
        nc.vector.tensor_tensor(out=zsum[:], in0=zacc[:, 0:1], in1=zacc[:, 1:2], op=OP.add)
        z2 = small.tile([64, 2], F32, tag="z2")
        nc.scalar.dma_start(z2[:], zsum[:])
        Zs = small.tile([64, 1], F32, tag="Zs")
        nc.vector.tensor_tensor(out=Zs[:], in0=z2[:, 0:1], in1=z2[:, 1:2], op=OP.add)
        rz = small.tile([64, 1], F32, tag="rz")
        nc.vector.reciprocal(rz[:], Zs[:])
        ptgt = small.tile([64, 1], F32, tag="ptgt")
        nc.vector.tensor_tensor(out=ptgt[:], in0=eld[:], in1=rz[:], op=OP.mult)
        upd = small.tile([64, 1], F32, tag="upd")
        nc.vector.tensor_tensor(out=upd[:], in0=u_sb, in1=pd_sb[:], op=OP.mult)
        acc_r = small.tile([64, 1], F32, tag="acc_r")
        nc.vector.tensor_tensor(out=acc_r[:], in0=ptgt[:], in1=upd[:], op=OP.is_ge)
        pdpos = small.tile([64, 1], F32, tag="pdpos")
        nc.vector.tensor_scalar(out=pdpos[:], in0=pd_sb[:], scalar1=0.0, scalar2=None, op0=OP.is_gt)
        nc.vector.tensor_tensor(out=acc_r[:], in0=acc_r[:], in1=pdpos[:], op=OP.mult)

        # rejection-prefix logic over [8 pos, 16 req]
        acc16 = small.tile([8, 16], F32, tag="acc16")
        nc.scalar.dma_start(acc16[:, 0:8], acc_g[:])
        nc.scalar.dma_start(acc16[:, 8:16], acc_r[:])
        rejN = small.tile([8, 16], F32, tag="rejN")
        nc.vector.scalar_tensor_tensor(out=rejN[:], in0=acc16[:], scalar=1.0, in1=valid9_sb[0:8, :], op0=OP.subtract, op1=OP.mult)
        rb_ps = psum.tile([9, 16], F32, tag="rb", space="PSUM")
        nc.tensor.matmul(rb_ps[:], lhsT=U9_sb, rhs=rejN[:], start=True, stop=True)
        keep = small.tile([9, 16], F32, tag="keep")
        nc.vector.tensor_scalar(out=keep[:], in0=rb_ps[:], scalar1=0.0, scalar2=None, op0=OP.is_equal)
        fr = small.tile([8, 16], F32, tag="fr")
        nc.vector.tensor_tensor(out=fr[:], in0=keep[0:8, :], in1=rejN[:], op=OP.mult)
        nc.vector.tensor_scalar(out=fr[:], in0=fr[:], scalar1=-1.0, scalar2=None, op0=OP.mult)
        accM = small.tile([8, 8], F32, tag="accM")
        nc.vector.tensor_scalar(out=accM[:], in0=rejN[:, 8:16], scalar1=1.0, scalar2=None, op0=OP.add)

        # selected-row subrow offsets: 128*sel_pos[m//16] + m, via matmul with
        # a column-replicated lhsT (lhsT[p, m] = fr[p, 8 + m//16])
        fr_rep = fr[:, 8:16].rearrange("p (j o) -> p j o", o=1).to_broadcast([8, 8, 16])
        frx = small.tile([8, 128], F32, tag="frx")
        nc.vector.tensor_copy(frx[:], fr_rep)
        os_ps = psum.tile([128, 1], F32, tag="osps", space="PSUM")
        nc.tensor.matmul(os_ps[:], lhsT=frx[:], rhs=c128_sb, start=True, stop=True)
        offsub_f = small.tile([128, 1], F32, tag="offsub_f")
        nc.vector.tensor_tensor(out=offsub_f[:], in0=os_ps[:], in1=pmfull_sb, op=OP.add)
        offsub_i = small.tile([128, 1], I32, tag="offsub_i")
        nc.vector.tensor_copy(offsub_i[:], offsub_f[:])

        # selected-row softmax denominator, broadcast to its 16 partitions
        Zs8 = small.tile([8, 8], F32, tag="Zs8")
        nc.scalar.dma_start(Zs8[:], Zs[:])
        zfr = small.tile([8, 8], F32, tag="zfr")
        nc.vector.tensor_tensor(out=zfr[:], in0=Zs8[:], in1=fr[:, 8:16], op=OP.mult)
        zfr_rep = zfr[:].rearrange("p (j o) -> p j o", o=1).to_broadcast([8, 8, 16])
        zfx = small.tile([8, 128], F32, tag="zfx")
        nc.vector.tensor_copy(zfx[:], zfr_rep)
        zb_ps = psum.tile([128, 1], F32, tag="zbps", space="PSUM")
        nc.tensor.matmul(zb_ps[:], lhsT=zfx[:], rhs=ones8_sb, start=True, stop=True)

        # gather the first-rejection row of each non-greedy request
        lsel = big.tile([128, CHW], F32, tag="lg")
        nc.gpsimd.indirect_dma_start(
            out=lsel[:, :SUB], out_offset=None, in_=lr_sub[:],
            in_offset=bass.IndirectOffsetOnAxis(ap=offsub_i[:, :1], axis=0))
        dsel = big.tile([128, CHW], F32, tag="lg")
        nc.gpsimd.indirect_dma_start(
            out=dsel[:, :SUB], out_offset=None, in_=dr_sub[:],
            in_offset=bass.IndirectOffsetOnAxis(ap=offsub_i[:, :1], axis=0))

        # recovered-token argmax over (exp(l) - Z d) / q  (sign-correct, q<=0 loses)
        esel = lrp.tile([128, CHW], F32, tag="lr")
        nc.scalar.activation(esel[:, :SUB], lsel[:, :SUB], AF.Exp)
        s_t = lrp.tile([128, CHW], F32, tag="lr")
        nc.vector.scalar_tensor_tensor(out=s_t[:, :SUB], in0=dsel[:, :SUB], scalar=zb_ps[:], in1=esel[:, :SUB], op0=OP.mult, op1=OP.subtract)
        adj = big.tile([128, CHW], F32, tag="lg")
        nc.vector.tensor_tensor(out=adj[:, :SUB], in0=s_t[:, :SUB], in1=rqs[:], op=OP.mult)
        rmax8 = small.tile([128, 8], F32, tag="rmax8")
        nc.vector.max(out=rmax8[:], in_=adj[:, :SUB])
        ridx8 = small.tile([128, 8], U32, tag="ridx8")
        nc.vector.max_index(ridx8[:], rmax8[:], adj[:, :SUB])
        rpk = small.tile([128, 2], F32, tag="rpk")
        nc.vector.tensor_copy(rpk[:, 1:2], ridx8[:, 0:1])
        nc.vector.tensor_tensor(out=rpk[:, 1:2], in0=rpk[:, 1:2], in1=off2000_sb, op=OP.add)
        nc.vector.tensor_copy(rpk[:, 0:1], rmax8[:, 0:1])
        rpk16 = small.tile([8, 32], F32, tag="rpk16")
        nc.scalar.dma_start(rpk16[:], rpk[:])
        jm8 = small.tile([8, 8], F32, tag="jm8")
        nc.vector.max(out=jm8[:], in_=rpk16[:, 0:32:2])
        jidx8 = small.tile([8, 8], U32, tag="jidx8")
        nc.vector.max_index(jidx8[:], jm8[:], rpk16[:, 0:32:2])
        jsf = small.tile([8, 1], F32, tag="jsf")
        nc.vector.tensor_copy(jsf[:], jidx8[:, 0:1])
        msel = small.tile([8, 16], F32, tag="msel")
        nc.vector.tensor_scalar(out=msel[:], in0=jiota16_sb, scalar1=jsf[:], scalar2=None, op0=OP.is_equal)
        nc.vector.tensor_tensor(out=msel[:], in0=msel[:], in1=rpk16[:, 1:32:2], op=OP.mult)
        recov8 = small.tile([8, 1], F32, tag="recov8")
        nc.vector.tensor_reduce(out=recov8[:], in_=msel[:], op=OP.add, axis=AX.X)
        recovrow = small.tile([1, 8], F32, tag="recovrow")
        nc.scalar.dma_start(recovrow[:], recov8[:])
        ones18 = small.tile([1, 8], F32, tag="ones18")
        nc.vector.memset(ones18[:], 1.0)
        rb2_ps = psum.tile([8, 8], F32, tag="rbc", space="PSUM")
        nc.tensor.matmul(rb2_ps[:], lhsT=ones18[:], rhs=recovrow[:], start=True, stop=True)

        # final assembly of the [9 pos, 16 req] output
        cand = small.tile([9, 16], F32, tag="cand")
        nc.vector.memset(cand[:], 0.0)
        nc.scalar.dma_start(cand[0:8, 0:8], amx[:])
        t1 = small.tile([8, 8], F32, tag="t1")
        nc.vector.tensor_tensor(out=t1[:], in0=accM[:], in1=didr8_sb, op=OP.mult)
        invA = small.tile([8, 8], F32, tag="invA")
        nc.vector.tensor_scalar(out=invA[:], in0=accM[:], scalar1=-1.0, scalar2=1.0, op0=OP.mult, op1=OP.add)
        nc.vector.tensor_tensor(out=invA[:], in0=invA[:], in1=rb2_ps[:], op=OP.mult)
        nc.vector.tensor_tensor(out=cand[0:8, 8:16], in0=t1[:], in1=invA[:], op=OP.add)

        w1 = small.tile([9, 16], F32, tag="w1")
        nc.vector.tensor_tensor(out=w1[:], in0=keep[:], in1=valid9_sb, op=OP.mult)
        w2 = small.tile([9, 16], F32, tag="w2")
        nc.vector.tensor_tensor(out=w2[:], in0=keep[:], in1=bonus_oh_sb, op=OP.mult)
        outf = small.tile([9, 16], F32, tag="outf")
        nc.vector.tensor_tensor(out=outf[:], in0=w1[:], in1=cand[:], op=OP.mult)
        ob = small.tile([9, 16], F32, tag="ob")
        nc.vector.tensor_tensor(out=ob[:], in0=w2[:], in1=bonusR_sb, op=OP.mult)
        nc.vector.tensor_tensor(out=outf[:], in0=outf[:], in1=ob[:], op=OP.add)
        nc.vector.tensor_tensor(out=outf[:], in0=outf[:], in1=w1[:], op=OP.add)
        nc.vector.tensor_tensor(out=outf[:], in0=outf[:], in1=w2[:], op=OP.add)
        nc.vector.tensor_scalar(out=outf[:], in0=outf[:], scalar1=1.0, scalar2=None, op0=OP.subtract)
        outi = small.tile([9, 16], I32, tag="outi")
        nc.vector.tensor_copy(outi[:], outf[:])
        nc.scalar.dma_start(out_e[:], outi[:])

    nc.compile()
    return nc


def _host_prepare(inputs):
    dp = np.ascontiguousarray(np.asarray(inputs["draft_probs"], np.float32))
    tl = np.ascontiguousarray(np.asarray(inputs["target_logits"], np.float32))
    q = np.ascontiguousarray(np.asarray(inputs["q"], np.float32))
    u = np.asarray(inputs["uniform_probs"], np.float32)
    temp = np.asarray(inputs["temperature"], np.float32)
    did = np.asarray(inputs["draft_token_ids"], np.int32)
    cu = np.asarray(inputs["cu_num_draft_tokens"], np.int64)
    bonus = np.asarray(inputs["bonus_token_ids"], np.int32)

    prev_cu = np.concatenate([[0], cu[:-1]])
    nd = cu - prev_cu
    if nd.min() < 0 or nd.max() > SPEC:
        raise _FallbackNeeded()
    is_greedy = temp == GREEDY_TEMPERATURE

    pgrid = np.arange(SPEC)

    # shared metadata
    m128 = np.stack([
        np.arange(128, dtype=np.float32) % 16 * SUB,      # off2000
        np.arange(128, dtype=np.float32),                 # pmfull
        np.zeros(128, np.float32),                        # spare
    ], axis=1)
    in_maps = []
    colmaps = []
    for c in range(NCORES):
        reqs = np.arange(c * RPC, (c + 1) * RPC)
        G = reqs[is_greedy[reqs]]
        R = reqs[~is_greedy[reqs]]
        if len(G) != 8 or len(R) != 8:
            raise _FallbackNeeded()
        cols = np.concatenate([G, R])
        colmaps.append(cols)

        tokG = prev_cu[G][None, :] + pgrid[:, None]
        mG = pgrid[:, None] < nd[G][None, :]
        tokG = np.where(mG, tokG, -1).reshape(64)
        tokR = prev_cu[R][None, :] + pgrid[:, None]
        mR = pgrid[:, None] < nd[R][None, :]
        tokR = np.where(mR, tokR, -1).reshape(64)

        lgrows = np.where((tokG >= 0)[:, None], tl[tokG.clip(0)], 0.0)
        lrrows = np.where((tokR >= 0)[:, None], tl[tokR.clip(0)], 0.0)
        drrows = np.where((tokR >= 0)[:, None], dp[tokR.clip(0)], 0.0)

        didg = np.where(tokG >= 0, did[tokG.clip(0)], 0).astype(np.float32)
        didr = np.where(tokR >= 0, did[tokR.clip(0)], 0)
        u_s = np.where(tokR >= 0, u[tokR.clip(0)], 0.0).astype(np.float32)
        offs = (np.arange(64) * V + didr).astype(np.int32)

        valid9 = (np.arange(9)[:, None] < nd[cols][None, :]).astype(np.float32)
        bonus_oh = (np.arange(9)[:, None] == nd[cols][None, :]).astype(np.float32)
        bonusR = np.broadcast_to(bonus[cols, 0].astype(np.float32)[None, :], (9, 16))
        m9 = np.concatenate([valid9, bonus_oh, bonusR], axis=1).astype(np.float32)

        U9 = np.triu(np.ones((9, 9), np.float32), 1)[:8]
        m8 = np.concatenate([
            didr.astype(np.float32).reshape(8, 8),        # 0:8   didr8
            U9,                                           # 8:17
            np.ones((8, 1), np.float32),                  # 17:18 ones8
            np.tile(np.arange(16, dtype=np.float32), (8, 1)),  # 18:34 jiota16
            128.0 * np.arange(8, dtype=np.float32)[:, None],   # 34:35 c128
            np.zeros((8, 1), np.float32),                 # 35:36 spare
        ], axis=1)
        m64 = np.stack([u_s, didg], axis=1).astype(np.float32)

        in_maps.append({
            "lg": lgrows.reshape(128, HALF),
            "lr": lrrows.reshape(128, HALF),
            "dr": drrows,
            "qr": np.ascontiguousarray(q[R]),
            "m128": m128,
            "m9": m9,
            "m8": m8,
            "m64": m64,
            "offs": offs[:, None],
        })
    return in_maps, colmaps


class _FallbackNeeded(Exception):
    pass


def _numpy_reference(inputs):
    """Pure-numpy port of the reference; fallback for unexpected input shapes."""
    dp = np.asarray(inputs["draft_probs"], np.float32)
    tl = np.asarray(inputs["target_logits"], np.float32)
    q = np.asarray(inputs["q"], np.float32)
    u = np.asarray(inputs["uniform_probs"], np.float32)
    temp = np.asarray(inputs["temperature"], np.float32)
    did = np.asarray(inputs["draft_token_ids"], np.int32)
    cu = np.asarray(inputs["cu_num_draft_tokens"], np.int64)
    bonus = np.asarray(inputs["bonus_token_ids"], np.int32)
    msl = int(np.asarray(inputs["max_spec_len"]))
    n = did.shape[0]
    b = cu.shape[0]
    x = tl - tl.max(axis=-1, keepdims=True)
    e = np.exp(x)
    tp = e / e.sum(axis=-1, keepdims=True)
    tok = np.arange(n)
    req = np.searchsorted(cu, tok, side="right")
    prev = np.concatenate([[0], cu[:-1]])
    start = prev[req]
    pos = tok - start
    g = (temp == GREEDY_TEMPERATURE)[req]
    am = tp.argmax(axis=-1).astype(np.int32)
    accg = did == am
    pd = dp[tok, did]
    pt = tp[tok, did]
    accr = (pd > 0) & (pt >= u * pd)
    adjusted = np.maximum(tp - dp, 0.0)
    rec = (adjusted / q[req]).argmax(axis=-1).astype(np.int32)
    acc = np.where(g, accg, accr)
    token = np.where(g, am, np.where(accr, did, rec))
    rej = (~acc).astype(np.int64)
    cs = rej.cumsum()
    seg = np.where(start > 0, cs[(start - 1).clip(0)], 0)
    keep = (cs - seg - rej) == 0
    out = np.full((b, msl + 1), PLACEHOLDER, np.int32)
    out[req, pos] = np.where(keep, token, PLACEHOLDER)
    ndr = cu - prev
    segrej = np.zeros(b, np.int64)
    np.add.at(segrej, req, rej)
    out[np.arange(b), ndr] = np.where(segrej == 0, bonus[:, 0], PLACEHOLDER)
    return out


def _run_on_cores(in_maps, trace=False):
    from concourse.bass_utils import run_bass_kernel_spmd
    if "nc" not in _NC_CACHE:
        _NC_CACHE["nc"] = _build()
    nc = _NC_CACHE["nc"]
    res = run_bass_kernel_spmd(nc, in_maps, core_ids=list(range(NCORES)), trace=trace)
    return res


def kernel(**inputs) -> np.ndarray:
    try:
        in_maps, colmaps = _host_prepare(inputs)
    except _FallbackNeeded:
        return _numpy_reference(inputs)
    res = _run_on_cores(in_maps, trace=False)
    out = np.full((B, SPEC + 1), PLACEHOLDER, np.int32)
    for c in range(NCORES):
        o = res.results[c]["out"]                # [9, 16] int32
        out[colmaps[c]] = o.T
    return out
